# revision 1
# baseline (speedup 1.0000x reference)
"""Trainium2 Bass kernel for DigitConvolutionalModel.

Model: x[B,784] -> reshape 28x28 -> 3x3 valid conv -> [B,676] -> FC(676,300)
       -> ReLU -> FC(300,10).

Strategy:
  * Fold the conv into FC1 on the host: feat @ w1 == x @ W1e where
    W1e[784,300] = C @ w1 (C = sparse conv scatter). Weight-only preprocessing.
  * Pure data parallel over 8 NeuronCores: batch shard of 8192 rows per core.
  * Per-core shard is passed pre-transposed (feature-major) so the contraction
    dim (784 = 7 chunks x 112) sits on SBUF partitions; the kernel computes
    transposed activations throughout (batch on the free axis):
        a1T[300,b] = relu(W1e.T @ xT + b1);  yT[10,b] = w2.T @ a1T + b2
    Biases are per-partition -> fold into scalar-engine activation bias.
  * float16 matmul operands (default): full-rate PE streaming, halved HBM
    traffic for x and halved weight loads, fp32 PSUM accumulation
    (~4e-4 rel err vs the fp32 reference; gate is 2e-2).
  * SUBT=2 (default): two batch subtiles share each stationary (j,k) weight
    load, shaving per-matmul weight-load overhead.
  * Output yT[10,8192] per core, un-transposed/gathered on host.

Measured per-pass steady state (8 cores, batch 65536): ~69.5us, vs ~83us for
the f32r/SUBT=1 baseline; the PE streaming floor for this decomposition
((7x3 + 3) matmuls x 512 cols x 16 tiles at ~2.8GHz) is ~69us, so the PE is
effectively saturated. An alternative layout (batch on PSUM partitions with
PE-transposes, LAYOUT=B env) is ~7% better on paper but ~7x slower on real
hardware (transpose/stationary-reload costs the model doesn't capture).
"""

import os
import sys

sys.path.insert(0, "/opt/trn_rl_repo")

import numpy as np

import concourse.tile as tile
from concourse import bacc, mybir
from concourse.bass_utils import run_bass_kernel_spmd

# ---- problem constants (hardcoded per harness contract) ----
B = 65536
D = 784  # 28*28
H = 300
O = 10
IMG = 28
KH = KW = 3
OUT_HW = IMG - KH + 1  # 26

N_CORES = 8
BS = B // N_CORES  # 8192 rows per core

KCH = 7  # contraction chunks
KP = D // KCH  # 112 partitions per chunk
BT = int(os.environ.get("BT_SIZE", "512"))  # batch tile (512 = one PSUM bank)
NBT = BS // BT
MPAD = 128  # padded partition count for hidden-chunk tensors

# matmul operand dtype: f32 (exact) | f32r (fast fp32 mode) | f16 (half inputs)
# f16 default: halves x HBM traffic + weight-load bytes; PE streams at the
# same 1 col/cycle, PSUM accumulation stays fp32 (~4e-4 rel err end-to-end).
_MM_CHOICE = os.environ.get("BASS_MM_DT", "f16")
MM_DT = {"f32": mybir.dt.float32, "f16": mybir.dt.float16}.get(
    _MM_CHOICE, mybir.dt.float32r
)
MM_NP = np.float16 if _MM_CHOICE == "f16" else np.float32

# hidden-dim chunking (sum must be H)
if os.environ.get("M_UNEVEN", "0") == "1":
    M_CHUNKS = [128, 128, 44]
else:
    M_CHUNKS = [100, 100, 100]
M_OFFS = [sum(M_CHUNKS[:i]) for i in range(len(M_CHUNKS))]
MCH = len(M_CHUNKS)

# layout: "A" = weights-stationary L1 (a1T native, 21+3 matmuls x 512 cols)
#         "B" = x-stationary L1 (batch on PSUM partitions, exact 300-col
#               streams) + PE transpose of a1 + exact-stream L2
LAYOUT = os.environ.get("LAYOUT", "A")
KPB = 113  # layoutB: 112 features + 1 ones-row (bias fold) on chunk 0
NBC = 4  # layoutB: 128-row batch chunks per 512 tile

# tunables (env-overridable for experiments)
# SUBT=2 default: two batch tiles share each stationary (j,k) weight load,
# measured ~2% faster than SUBT=1 on hardware (69.5us vs 71.1us per pass).
SUBT_DEFAULT = int(os.environ.get("SUBT", "2"))
XP_BUFS = int(os.environ.get("XP_BUFS", "3" if SUBT_DEFAULT <= 1 else "1"))
AP_BUFS = int(os.environ.get("AP_BUFS", "3" if SUBT_DEFAULT <= 1 else "2"))
PS1_BUFS = int(os.environ.get("PS1_BUFS", ("4" if BT <= 512 else "3") if SUBT_DEFAULT <= 1 else "2"))
PS2_BUFS = int(os.environ.get("PS2_BUFS", "2" if BT <= 512 else "1"))
X_DMA_SPLIT = int(os.environ.get("X_DMA_SPLIT", "1"))  # k-chunk granularity of x loads
X_LAYOUT = os.environ.get("X_LAYOUT", "bt")  # "bt": batch-tile-major (contiguous loads); "k": k-major
REPS = int(os.environ.get("KERNEL_REPS", "1"))  # timing only: repeat body in-module
SUBT = SUBT_DEFAULT  # batch subtiles sharing one weight load

_cache = {}


def _build_nc():
    f32 = mybir.dt.float32
    mdt = MM_DT

    nc = bacc.Bacc("TRN2", target_bir_lowering=False, debug=False, num_devices=N_CORES)
    if X_LAYOUT == "bt":
        xt_d = nc.declare_dram_parameter("xt", [KP, NBT, KCH, BT], mdt, isOutput=False)
    else:
        xt_d = nc.declare_dram_parameter("xt", [KP, KCH, BS], mdt, isOutput=False)
    w1_d = nc.declare_dram_parameter("w1e", [KP, KCH * H], mdt, isOutput=False)
    b1_d = nc.declare_dram_parameter("b1r", [MPAD, MCH], f32, isOutput=False)
    w2_d = nc.declare_dram_parameter("w2r", [MPAD, MCH * O], mdt, isOutput=False)
    b2_d = nc.declare_dram_parameter("b2r", [O, 1], f32, isOutput=False)
    yt_d = nc.declare_dram_parameter("yt", [O, BS], f32, isOutput=True)

    with tile.TileContext(nc) as tc:
        with (
            tc.tile_pool(name="singles", bufs=1) as singles,
            tc.tile_pool(name="xp", bufs=XP_BUFS) as xp,
            tc.tile_pool(name="ap", bufs=AP_BUFS) as ap,
            tc.tile_pool(name="yp", bufs=3) as yp,
            tc.tile_pool(name="ps1", bufs=PS1_BUFS, space="PSUM") as ps1p,
            tc.tile_pool(name="ps2", bufs=PS2_BUFS, space="PSUM") as ps2p,
        ):
            w1sb = singles.tile([KP, KCH * H], mdt)
            nc.sync.dma_start(w1sb[:], w1_d[:])
            b1sb = singles.tile([MPAD, MCH], f32)
            nc.sync.dma_start(b1sb[:], b1_d[:])
            w2sb = singles.tile([MPAD, MCH * O], mdt)
            nc.sync.dma_start(w2sb[:], w2_d[:])
            b2sb = singles.tile([O, 1], f32)
            nc.sync.dma_start(b2sb[:], b2_d[:])

            def load_x(bt):
                tag = "xt" if SUBT <= 1 else f"xt{bt % (SUBT + 2)}"
                xt = xp.tile([KP, KCH, BT], mdt, name=tag)
                step = (KCH + X_DMA_SPLIT - 1) // X_DMA_SPLIT if X_DMA_SPLIT > 1 else KCH
                for s in range(0, KCH, step):
                    e = min(s + step, KCH)
                    if X_LAYOUT == "bt":
                        nc.sync.dma_start(xt[:, s:e, :], xt_d[:, bt, s:e, :])
                    else:
                        nc.sync.dma_start(
                            xt[:, s:e, :],
                            xt_d[:, s:e, bt * BT : (bt + 1) * BT],
                        )
                return xt

            def layer2_store(a1, bt):
                ps2 = ps2p.tile([O, BT], f32)
                for j in range(MCH):
                    mlen = M_CHUNKS[j]
                    nc.tensor.matmul(
                        ps2[:],
                        w2sb[0:mlen, j * O : (j + 1) * O],
                        a1[0:mlen, j, :],
                        start=(j == 0),
                        stop=(j == MCH - 1),
                    )
                yt = yp.tile([O, BT], f32)
                nc.vector.tensor_scalar_add(yt[:], ps2[:], b2sb[:, 0:1])
                nc.sync.dma_start(yt_d[:, bt * BT : (bt + 1) * BT], yt[:])

            if SUBT <= 1:
                L2_DEFER = int(os.environ.get("L2_DEFER", "0"))
                pending = None
                for bt in [i for _ in range(REPS) for i in range(NBT)]:
                    xt = load_x(bt)
                    a1 = ap.tile([MPAD, MCH, BT], mdt)
                    for j in range(MCH):
                        mlen, moff = M_CHUNKS[j], M_OFFS[j]
                        ps = ps1p.tile([MPAD, BT], f32)
                        for k in range(KCH):
                            nc.tensor.matmul(
                                ps[0:mlen, :],
                                w1sb[:, k * H + moff : k * H + moff + mlen],
                                xt[:, k, :],
                                start=(k == 0),
                                stop=(k == KCH - 1),
                            )
                        if j == 1 and L2_DEFER and pending is not None:
                            # defer the previous tile's L2 behind this tile's
                            # first matmul groups so the PE never waits on
                            # that tile's last ACT eviction
                            layer2_store(*pending)
                            pending = None
                        nc.scalar.activation(
                            a1[0:mlen, j, :],
                            ps[0:mlen, :],
                            mybir.ActivationFunctionType.Relu,
                            bias=b1sb[0:mlen, j : j + 1],
                        )
                    if L2_DEFER:
                        pending = (a1, bt)
                    else:
                        layer2_store(a1, bt)
                if pending is not None:
                    layer2_store(*pending)
            else:
                # weight-reuse grouping: SUBT batch subtiles per (j,k) stationary
                for g in [i for _ in range(REPS) for i in range(NBT // SUBT)]:
                    bts = [g * SUBT + s for s in range(SUBT)]
                    xts = [load_x(bt) for bt in bts]
                    a1s = [
                        ap.tile([MPAD, MCH, BT], mdt, name=f"a1{s}")
                        for s in range(SUBT)
                    ]
                    for j in range(MCH):
                        mlen, moff = M_CHUNKS[j], M_OFFS[j]
                        pss = [
                            ps1p.tile([MPAD, BT], f32, name=f"ps{s}")
                            for s in range(SUBT)
                        ]
                        for k in range(KCH):
                            for s in range(SUBT):
                                nc.tensor.matmul(
                                    pss[s][0:mlen, :],
                                    w1sb[:, k * H + moff : k * H + moff + mlen],
                                    xts[s][:, k, :],
                                    start=(k == 0),
                                    stop=(k == KCH - 1),
                                )
                        for s in range(SUBT):
                            nc.scalar.activation(
                                a1s[s][0:mlen, j, :],
                                pss[s][0:mlen, :],
                                mybir.ActivationFunctionType.Relu,
                                bias=b1sb[0:mlen, j : j + 1],
                            )
                    for s in range(SUBT):
                        layer2_store(a1s[s], bts[s])

    nc.compile()
    return nc


def _build_nc_b():
    """layoutB: L1 streams w1e (300 cols exactly) with x stationary; a1 comes
    out [batch, h], PE-transposed back to [h, batch] for L2. Biases are folded
    as an extra contraction row (ones in the activations, bias in the
    weights), since the ACT-engine bias is per-partition and both biases here
    live on the free dim at their application point."""
    f32 = mybir.dt.float32
    mdt = MM_DT
    assert BT == 512, "layoutB assumes BT=512 (4x128 batch chunks)"

    nc = bacc.Bacc("TRN2", target_bir_lowering=False, debug=False, num_devices=N_CORES)
    xt_d = nc.declare_dram_parameter("xt", [KPB, NBT, KCH, BT], mdt, isOutput=False)
    w1_d = nc.declare_dram_parameter("w1e", [KPB, KCH * H], mdt, isOutput=False)
    w2_d = nc.declare_dram_parameter("w2r", [101, MCH * O], mdt, isOutput=False)
    id_d = nc.declare_dram_parameter("ident", [128, 128], mdt, isOutput=False)
    ones_d = nc.declare_dram_parameter("ones", [1, BT], mdt, isOutput=False)
    yt_d = nc.declare_dram_parameter("yt", [O, BS], f32, isOutput=True)

    ATP_BUFS = int(os.environ.get("ATP_BUFS", "2"))
    ASB_BUFS = int(os.environ.get("ASB_BUFS", "3"))
    PSA_BUFS = int(os.environ.get("PSA_BUFS", "2"))
    PST_BUFS = int(os.environ.get("PST_BUFS", "3"))
    TCOPY_SPLIT = int(os.environ.get("TCOPY_SPLIT", "1"))

    with tile.TileContext(nc) as tc:
        with (
            tc.tile_pool(name="singles", bufs=1) as singles,
            tc.tile_pool(name="xp", bufs=XP_BUFS) as xp,
            tc.tile_pool(name="asb", bufs=ASB_BUFS) as asb,
            tc.tile_pool(name="atp", bufs=ATP_BUFS) as atp,
            tc.tile_pool(name="yp", bufs=3) as yp,
            tc.tile_pool(name="psA", bufs=PSA_BUFS, space="PSUM") as psA,
            tc.tile_pool(name="psT", bufs=PST_BUFS, space="PSUM") as psT,
            tc.tile_pool(name="ps2", bufs=PS2_BUFS, space="PSUM") as ps2p,
        ):
            w1sb = singles.tile([KPB, KCH * H], mdt)
            nc.sync.dma_start(w1sb[:], w1_d[:])
            w2sb = singles.tile([101, MCH * O], mdt)
            nc.sync.dma_start(w2sb[:], w2_d[:])
            idsb = singles.tile([128, 128], mdt)
            nc.sync.dma_start(idsb[:], id_d[:])

            pending_l2 = [None]

            def emit_T(a1T, pa1sb, pc):
                """PE-transpose one relu'd batch chunk into a1T; evict copies
                alternate DVE/Pool so neither engine serializes the chain."""
                for j in range(MCH):
                    tps = psT.tile([100, 128], mdt)
                    nc.tensor.transpose(
                        tps[:], pa1sb[:, j * 100 : (j + 1) * 100], idsb[:]
                    )
                    dst = a1T[0:100, j, pc * 128 : (pc + 1) * 128]
                    if TCOPY_SPLIT and j % 2:
                        nc.scalar.activation(
                            dst, tps[:], mybir.ActivationFunctionType.Copy
                        )
                    else:
                        nc.vector.tensor_copy(dst, tps[:])

            def emit_l2(a1T, bt):
                ps2 = ps2p.tile([O, BT], f32)
                for j in range(MCH):
                    kk = 101 if j == 0 else 100
                    nc.tensor.matmul(
                        ps2[:],
                        w2sb[0:kk, j * O : (j + 1) * O],
                        a1T[0:kk, j, :],
                        start=(j == 0),
                        stop=(j == MCH - 1),
                    )
                yt = yp.tile([O, BT], f32)
                nc.vector.tensor_copy(yt[:], ps2[:])
                nc.sync.dma_start(yt_d[:, bt * BT : (bt + 1) * BT], yt[:])

            for bt in [i for _ in range(REPS) for i in range(NBT)]:
                xt = xp.tile([KPB, KCH, BT], mdt)
                nc.sync.dma_start(xt[:], xt_d[:, bt, :, :])
                a1T = atp.tile([101, MCH, BT], mdt, name="a1T")
                # ones-row for the L2 bias fold (ACT bias is per-partition
                # only; b2 lives on the free dim at its application point)
                nc.sync.dma_start(a1T[100:101, 0, :], ones_d[:])
                prev = None
                for c in range(NBC):
                    a1ps = psA.tile([128, H], f32)
                    for k in range(KCH):
                        kk = KPB if k == 0 else KP
                        nc.tensor.matmul(
                            a1ps[:],
                            xt[0:kk, k, c * 128 : (c + 1) * 128],
                            w1sb[0:kk, k * H : (k + 1) * H],
                            start=(k == 0),
                            stop=(k == KCH - 1),
                        )
                    if c == 0 and pending_l2[0] is not None:
                        # previous tile's L2, emitted late so the PE queue
                        # never stalls on that tile's last transpose evict
                        emit_l2(*pending_l2[0])
                        pending_l2[0] = None
                    a1sb = asb.tile([128, H], mdt)
                    nc.scalar.activation(
                        a1sb[:], a1ps[:], mybir.ActivationFunctionType.Relu
                    )
                    if prev is not None:
                        emit_T(a1T, *prev)
                    prev = (a1sb, c)
                emit_T(a1T, *prev)
                pending_l2[0] = (a1T, bt)
            emit_l2(*pending_l2[0])
            pending_l2[0] = None

    nc.compile()
    return nc


def _host_prep_weights_b(conv_w, w1, b1, w2, b2):
    w1g = w1.astype(np.float64).reshape(OUT_HW, OUT_HW, H)
    w1e = np.zeros((IMG, IMG, H), dtype=np.float64)
    cw = conv_w.astype(np.float64)
    for di in range(KH):
        for dj in range(KW):
            w1e[di : di + OUT_HW, dj : dj + OUT_HW, :] += cw[di, dj] * w1g
    w1e = w1e.reshape(D, H).astype(np.float32)

    w1e_r = np.zeros((KPB, KCH * H), np.float32)
    for k in range(KCH):
        w1e_r[0:KP, k * H : (k + 1) * H] = w1e[k * KP : (k + 1) * KP, :]
    w1e_r[KP, 0:H] = b1.reshape(H)  # ones-row bias fold (chunk 0)

    w2_r = np.zeros((101, MCH * O), np.float32)
    for j in range(MCH):
        w2_r[0:100, j * O : (j + 1) * O] = w2[j * 100 : (j + 1) * 100, :]
    w2_r[100, 0:O] = b2.reshape(O)

    ident = np.eye(128, dtype=MM_NP)
    return w1e_r.astype(MM_NP), w2_r.astype(MM_NP), ident


def _host_prep_x_b(xc):
    """Per-core shard [BS, 784] -> [KPB, NBT, KCH, BT] with ones-row."""
    xc = xc.astype(MM_NP)
    xt = np.empty((KPB, NBT, KCH, BT), MM_NP)
    xt[0:KP] = xc.reshape(NBT, BT, KCH, KP).transpose(3, 0, 2, 1)
    xt[KP, :, 0, :] = 1.0
    xt[KP, :, 1:, :] = 0.0
    return xt


def _host_prep_weights(conv_w, w1, b1, w2, b2):
    # Fold conv into FC1: W1e = C @ w1, computed in f64 then cast.
    w1g = w1.astype(np.float64).reshape(OUT_HW, OUT_HW, H)
    w1e = np.zeros((IMG, IMG, H), dtype=np.float64)
    cw = conv_w.astype(np.float64)
    for di in range(KH):
        for dj in range(KW):
            w1e[di : di + OUT_HW, dj : dj + OUT_HW, :] += cw[di, dj] * w1g
    w1e = w1e.reshape(D, H).astype(np.float32)

    w1e_r = np.ascontiguousarray(
        w1e.reshape(KCH, KP, H).transpose(1, 0, 2).reshape(KP, KCH * H)
    ).astype(MM_NP)
    b1f = b1.reshape(H)
    b1_r = np.zeros((MPAD, MCH), np.float32)
    w2_r = np.zeros((MPAD, MCH * O), MM_NP)
    for j in range(MCH):
        mlen, moff = M_CHUNKS[j], M_OFFS[j]
        b1_r[0:mlen, j] = b1f[moff : moff + mlen]
        w2_r[0:mlen, j * O : (j + 1) * O] = w2[moff : moff + mlen, :]
    b2_r = np.ascontiguousarray(b2.reshape(O, 1))
    return w1e_r, b1_r, w2_r, b2_r


def _host_prep_x(xc):
    """Per-core shard [BS, 784] -> feature-major DRAM layout."""
    xc = xc.astype(MM_NP)
    if X_LAYOUT == "bt":
        # xt[p, bt, k, b] = xc[bt*BT + b, k*KP + p]: per-(partition, batch-tile)
        # loads are fully contiguous per partition.
        return np.ascontiguousarray(
            xc.reshape(NBT, BT, KCH, KP).transpose(3, 0, 2, 1)
        )
    # xt[p, k, b] = xc[b, k*KP + p]
    return np.ascontiguousarray(xc.T.reshape(KCH, KP, BS).transpose(1, 0, 2))


def make_in_maps(x, conv_w, w1, b1, w2, b2):
    """Full inputs -> per-core input maps for the current LAYOUT."""
    x = np.asarray(x, dtype=np.float32)
    conv_w = np.asarray(conv_w, np.float32)
    w1 = np.asarray(w1, np.float32)
    b1 = np.asarray(b1, np.float32)
    w2 = np.asarray(w2, np.float32)
    b2 = np.asarray(b2, np.float32)
    in_maps = []
    if LAYOUT == "B":
        w1e_r, w2_r, ident = _host_prep_weights_b(conv_w, w1, b1, w2, b2)
        ones = np.ones((1, BT), MM_NP)
        for c in range(N_CORES):
            xt = _host_prep_x_b(x[c * BS : (c + 1) * BS])
            in_maps.append(
                {"xt": xt, "w1e": w1e_r, "w2r": w2_r, "ident": ident,
                 "ones": ones}
            )
    else:
        w1e_r, b1_r, w2_r, b2_r = _host_prep_weights(conv_w, w1, b1, w2, b2)
        for c in range(N_CORES):
            xt = _host_prep_x(x[c * BS : (c + 1) * BS])
            in_maps.append(
                {"xt": xt, "w1e": w1e_r, "b1r": b1_r, "w2r": w2_r, "b2r": b2_r}
            )
    return in_maps


def build_nc():
    return _build_nc_b() if LAYOUT == "B" else _build_nc()


def kernel(x, conv_w, w1, b1, w2, b2):
    if "nc" not in _cache:
        _cache["nc"] = build_nc()
    nc = _cache["nc"]

    in_maps = make_in_maps(x, conv_w, w1, b1, w2, b2)
    res = run_bass_kernel_spmd(nc, in_maps, list(range(N_CORES)))

    y = np.empty((B, O), dtype=np.float32)
    for c in range(N_CORES):
        y[c * BS : (c + 1) * BS] = res.results[c]["yt"].T
    return y



# revision 2
# speedup vs baseline: 1.2483x; 1.2483x over previous
"""Trainium2 Bass kernel for DigitConvolutionalModel.

Model: x[B,784] -> reshape 28x28 -> 3x3 valid conv -> [B,676] -> FC(676,300)
       -> ReLU -> FC(300,10).

Strategy:
  * Fold the conv into FC1 on the host: feat @ w1 == x @ W1e where
    W1e[784,300] = C @ w1 (C = sparse conv scatter). Weight-only preprocessing.
  * Pure data parallel over 8 NeuronCores: batch shard of 8192 rows per core.
  * Per-core shard is passed pre-transposed (feature-major) so the contraction
    dim sits on SBUF partitions; the kernel computes transposed activations
    throughout (batch on the free axis):
        a1T[300,b] = relu(W1e.T @ xT + b1);  yT[10,b] = w2.T @ a1T + b2
  * float16 matmul operands, fp32 PSUM accumulation (~4e-4 rel err vs the
    fp32 reference; gate is 2e-2).
  * L1 contraction chunked 6x128 + 16: the three m-chunks' 16-row tail
    matmuls are issued as one concurrent row-tiled group (tile_position
    bases 0/32/64), so L1 costs 18+1 PE stream slots instead of 21; with
    L2's 3 slots the per-512-row-tile total is 22 slots (the packing floor
    for this decomposition) vs 24 for the previous 7x112 layout.
  * Tails issue FIRST in each accumulation group (start=True) so each
    m-chunk closes right after its full-chunk matmuls and ACT evictions
    pipeline; the previous group's FC2 is deferred into the middle of the
    current group so the PE never waits on an ACT eviction.
  * m-chunks are [128,128,44]: 128-column stationaries enable fast weight
    load (FWL needs NumWeights==128).
  * SUBT=2 batch subtiles share each stationary (j,k) weight load.
  * Output yT[10,8192] per core, un-transposed/gathered on host.
"""

import os
import sys

sys.path.insert(0, "/opt/trn_rl_repo")

import numpy as np

import concourse.tile as tile
from concourse import bacc, mybir
from concourse.bass_utils import run_bass_kernel_spmd

# ---- problem constants (hardcoded per harness contract) ----
B = 65536
D = 784  # 28*28
H = 300
O = 10
IMG = 28
KH = KW = 3
OUT_HW = IMG - KH + 1  # 26

N_CORES = 8
BS = B // N_CORES  # 8192 rows per core

KF = 6  # full 128-row contraction chunks
KTAIL = D - KF * 128  # 16
BT = 512  # batch tile (one PSUM bank)
NBT = BS // BT  # 16
MPAD = 128

MM_DT = mybir.dt.float16
MM_NP = np.float16

if os.environ.get("M_UNEVEN", "1") == "1":
    M_CHUNKS = [128, 128, 44]  # 128-col stationaries trigger FWL
else:
    M_CHUNKS = [100, 100, 100]
M_OFFS = [sum(M_CHUNKS[:i]) for i in range(3)]
MCH = 3

SUBT = int(os.environ.get("SUBT", "2"))
XP_BUFS = int(os.environ.get("XP_BUFS", "2"))
AP_BUFS = int(os.environ.get("AP_BUFS", "2"))
PS2_BUFS = int(os.environ.get("PS2_BUFS", "2"))
TAIL_MODE = os.environ.get("TAIL_MODE", "pack")
REPS = int(os.environ.get("KERNEL_REPS", "1"))  # timing only

_cache = {}


def _build_nc():
    f32 = mybir.dt.float32
    mdt = MM_DT

    nc = bacc.Bacc("TRN2", target_bir_lowering=False, debug=False, num_devices=N_CORES)
    # x full chunks: [128, NBT, KF, BT]; tail replicated at partition bases
    # 0/32/64 with zero gaps: [80, NBT, BT]
    xt_d = nc.declare_dram_parameter("xt", [128, NBT, KF, BT], mdt, isOutput=False)
    xtl_d = nc.declare_dram_parameter("xtl", [80, NBT, BT], mdt, isOutput=False)
    w1_d = nc.declare_dram_parameter("w1e", [128, KF * H], mdt, isOutput=False)
    wtl_d = nc.declare_dram_parameter("wtl", [80, 128], mdt, isOutput=False)
    b1_d = nc.declare_dram_parameter("b1r", [MPAD, MCH], f32, isOutput=False)
    w2_d = nc.declare_dram_parameter("w2r", [MPAD, MCH * O], mdt, isOutput=False)
    b2_d = nc.declare_dram_parameter("b2r", [O, 1], f32, isOutput=False)
    yt_d = nc.declare_dram_parameter("yt", [O, BS], f32, isOutput=True)

    with tile.TileContext(nc) as tc:
        with (
            tc.tile_pool(name="singles", bufs=1) as singles,
            tc.tile_pool(name="xp", bufs=XP_BUFS) as xp,
            tc.tile_pool(name="xtp", bufs=XP_BUFS) as xtp,
            tc.tile_pool(name="ap", bufs=AP_BUFS) as ap,
            tc.tile_pool(name="yp", bufs=3) as yp,
            tc.tile_pool(name="ps1", bufs=(1 if SUBT == 2 else 2), space="PSUM") as ps1p,
            tc.tile_pool(name="ps2", bufs=PS2_BUFS, space="PSUM") as ps2p,
        ):
            w1sb = singles.tile([128, KF * H], mdt)
            nc.sync.dma_start(w1sb[:], w1_d[:])
            wtlsb = singles.tile([128, 128], mdt)
            nc.sync.dma_start(wtlsb[0:80, :], wtl_d[:])
            b1sb = singles.tile([MPAD, MCH], f32)
            nc.sync.dma_start(b1sb[:], b1_d[:])
            w2sb = singles.tile([MPAD, MCH * O], mdt)
            nc.sync.dma_start(w2sb[:], w2_d[:])
            b2sb = singles.tile([O, 1], f32)
            nc.sync.dma_start(b2sb[:], b2_d[:])

            def load_x(bt, s):
                xt = xp.tile([128, KF, BT], mdt, name=f"xt{s}")
                nc.sync.dma_start(xt[:], xt_d[:, bt, :, :])
                xtl = xtp.tile([128, BT], mdt, name=f"xtl{s}")
                nc.sync.dma_start(xtl[0:80, :], xtl_d[:, bt, :])
                return xt, xtl

            def layer2_store(a1, bt):
                ps2 = ps2p.tile([O, BT], f32)
                for j in range(MCH):
                    mlen = M_CHUNKS[j]
                    nc.tensor.matmul(
                        ps2[:],
                        w2sb[0:mlen, j * O : (j + 1) * O],
                        a1[0:mlen, j, :],
                        start=(j == 0),
                        stop=(j == MCH - 1),
                    )
                yt = yp.tile([O, BT], f32)
                nc.vector.tensor_scalar_add(yt[:], ps2[:], b2sb[:, 0:1])
                nc.sync.dma_start(yt_d[:, bt * BT : (bt + 1) * BT], yt[:])

            pending = []
            for g in [i for _ in range(REPS) for i in range(NBT // SUBT)]:
                bts = [g * SUBT + s for s in range(SUBT)]
                xts = [load_x(bt, s) for s, bt in enumerate(bts)]
                a1s = [
                    ap.tile([MPAD, MCH, BT], mdt, name=f"a1{s}") for s in range(SUBT)
                ]
                pss = [
                    [ps1p.tile([MPAD, BT], f32, name=f"ps{j}_{s}") for s in range(SUBT)]
                    for j in range(MCH)
                ]
                # 16-row tails FIRST (start=True): one concurrent row-tiled
                # group per subtile, so each m-chunk closes right after its
                # full-chunk matmuls and the ACT evictions pipeline.
                for s in range(SUBT):
                    for j in range(MCH):
                        base = j * 32
                        mlen = M_CHUNKS[j]
                        nc.tensor.matmul(
                            pss[j][s][0:mlen, :],
                            wtlsb[base : base + KTAIL, 0:mlen],
                            xts[s][1][base : base + KTAIL, :],
                            start=True,
                            stop=False,
                            tile_position=(base, 0) if TAIL_MODE == "pack" else None,
                        )
                # full-chunk matmuls: for each (j, k) the SUBT subtiles share
                # one stationary load; m-chunk j closes at k == KF-1.
                for j in range(MCH):
                    mlen, moff = M_CHUNKS[j], M_OFFS[j]
                    for k in range(KF):
                        for s in range(SUBT):
                            nc.tensor.matmul(
                                pss[j][s][0:mlen, :],
                                w1sb[:, k * H + moff : k * H + moff + mlen],
                                xts[s][0][:, k, :],
                                start=False,
                                stop=(k == KF - 1),
                            )
                    for s in range(SUBT):
                        nc.scalar.activation(
                            a1s[s][0:mlen, j, :],
                            pss[j][s][0:mlen, :],
                            mybir.ActivationFunctionType.Relu,
                            bias=b1sb[0:mlen, j : j + 1],
                        )
                    if j == 0 and pending:
                        # previous group's L2, emitted mid-group so the PE
                        # never waits on that group's last ACT eviction
                        for a1p, btp in pending:
                            layer2_store(a1p, btp)
                        pending = []
                pending = [(a1s[s], bts[s]) for s in range(SUBT)]
            for a1p, btp in pending:
                layer2_store(a1p, btp)

    nc.compile()
    return nc


def _host_prep_weights(conv_w, w1, b1, w2, b2):
    # Fold conv into FC1: W1e = C @ w1, computed in f64 then cast.
    w1g = w1.astype(np.float64).reshape(OUT_HW, OUT_HW, H)
    w1e = np.zeros((IMG, IMG, H), dtype=np.float64)
    cw = conv_w.astype(np.float64)
    for di in range(KH):
        for dj in range(KW):
            w1e[di : di + OUT_HW, dj : dj + OUT_HW, :] += cw[di, dj] * w1g
    w1e = w1e.reshape(D, H).astype(np.float32)

    w1e_r = np.ascontiguousarray(
        w1e[: KF * 128].reshape(KF, 128, H).transpose(1, 0, 2).reshape(128, KF * H)
    ).astype(MM_NP)
    wtl = np.zeros((80, 128), np.float32)
    for j in range(MCH):
        wtl[j * 32 : j * 32 + KTAIL, 0 : M_CHUNKS[j]] = w1e[
            KF * 128 :, M_OFFS[j] : M_OFFS[j] + M_CHUNKS[j]
        ]
    b1f = b1.reshape(H)
    b1_r = np.zeros((MPAD, MCH), np.float32)
    w2_r = np.zeros((MPAD, MCH * O), MM_NP)
    for j in range(MCH):
        mlen, moff = M_CHUNKS[j], M_OFFS[j]
        b1_r[0:mlen, j] = b1f[moff : moff + mlen]
        w2_r[0:mlen, j * O : (j + 1) * O] = w2[moff : moff + mlen, :]
    b2_r = np.ascontiguousarray(b2.reshape(O, 1))
    return w1e_r, wtl.astype(MM_NP), b1_r, w2_r, b2_r


def _host_prep_x(xc):
    """Per-core shard [BS, 784] -> full-chunk + replicated-tail layouts."""
    xc = xc.astype(MM_NP)
    xt = np.ascontiguousarray(
        xc[:, : KF * 128].reshape(NBT, BT, KF, 128).transpose(3, 0, 2, 1)
    )
    tail = xc[:, KF * 128 :].reshape(NBT, BT, KTAIL).transpose(2, 0, 1)
    xtl = np.zeros((80, NBT, BT), MM_NP)
    for j in range(MCH):
        xtl[j * 32 : j * 32 + KTAIL] = tail
    return xt, xtl


def make_in_maps(x, conv_w, w1, b1, w2, b2):
    """Full inputs -> per-core input maps."""
    x = np.asarray(x, dtype=np.float32)
    conv_w = np.asarray(conv_w, np.float32)
    w1 = np.asarray(w1, np.float32)
    b1 = np.asarray(b1, np.float32)
    w2 = np.asarray(w2, np.float32)
    b2 = np.asarray(b2, np.float32)
    w1e_r, wtl, b1_r, w2_r, b2_r = _host_prep_weights(conv_w, w1, b1, w2, b2)
    in_maps = []
    for c in range(N_CORES):
        xt, xtl = _host_prep_x(x[c * BS : (c + 1) * BS])
        in_maps.append(
            {
                "xt": xt,
                "xtl": xtl,
                "w1e": w1e_r,
                "wtl": wtl,
                "b1r": b1_r,
                "w2r": w2_r,
                "b2r": b2_r,
            }
        )
    return in_maps


def build_nc():
    return _build_nc()


def kernel(x, conv_w, w1, b1, w2, b2):
    if "nc" not in _cache:
        _cache["nc"] = build_nc()
    nc = _cache["nc"]

    in_maps = make_in_maps(x, conv_w, w1, b1, w2, b2)
    res = run_bass_kernel_spmd(nc, in_maps, list(range(N_CORES)))

    y = np.empty((B, O), dtype=np.float32)
    for c in range(N_CORES):
        y[c * BS : (c + 1) * BS] = res.results[c]["yt"].T
    return y
